# revision 34
# baseline (speedup 1.0000x reference)
"""Trainium2 Bass kernel for BasisOrbitalBackflow.

Math: for each batch b,
    basis[i, k*NB+l] = (1/(N-1)) * (u[i,k]) * chi[i,l],  u[i,k] = sum_{j != i} chi[j,k]
    out = basis @ W
(the mean over j != i of the pair outer product chi[j,k]*chi[i,l] collapses to
an outer product of the leave-one-out column sum with the row itself, so the
[B, N, N, NB^2] pair tensor is never materialized).

Sharding: data parallel, batch b -> core b (B == n_cores == 8).

Per-core dataflow (all shapes [partition, free]):
  u = (ONES - I) @ chi                                  (PE; one [128,128]x[128,32] matmul)
  t_ur[i, j*32+r] = u[i, j]                             (DVE/ACT stride-0 broadcast copies)
  urep_c[p, i] = t_ur_c.T = u[i, 4c + p>>5]             (PE transpose mode, fp32 single pass)
  vrep[p, i] = chi[i, p & 31]                           (same: tile 4x then PE transpose)
  bT[p, c*128+i] = urep_c[p, i] * vrep[p, i]            (DVE multiplies, chunk-pipelined)
  out[i, o] = sum_c sum_p bT_c[p, i] * w2[p, c*128+o]   (PE: 8 accumulating matmuls)
where w2[p, c*128+o] = W[c*128+p, o] / (N-1) is a host-side repack of
backflow_coeff (free: weights are static), and ONES-I / identity are built
on-device via memset + affine_select.
"""

import numpy as np

B, N, NB, NORB = 8, 128, 32, 128
NWARM = 0
# "fp32": full precision everywhere (rel err ~1e-6).
# "fp16": basis/W contraction in fp16 (single-pass PE, FWL) — rel err ~5e-4.
CONTRACT = "fp32"
COPY_ENGINE = "alt"

NB2 = NB * NB  # 1024
NCHUNK = NB2 // 128  # 8

_cache = {}


def _build():
    import concourse.bacc as bacc
    import concourse.mybir as mybir
    from concourse.tile import TileContext

    f32 = mybir.dt.float32
    fc = mybir.dt.float16 if CONTRACT == "fp16" else f32
    nc = bacc.Bacc(None, target_bir_lowering=False, debug=False, num_devices=8)
    chi = nc.declare_dram_parameter("chi", [N, NB], f32, isOutput=False)
    w2 = nc.declare_dram_parameter("w2", [128, NB2], fc, isOutput=False)
    y = nc.declare_dram_parameter("y", [NORB, N], f32, isOutput=True)

    with TileContext(nc) as tc:
        with (
            tc.tile_pool(name="sb", bufs=1) as pool,
            tc.tile_pool(name="ps", bufs=1, space="PSUM") as psum,
        ):
            # both loads on the sync HWDGE ring, chi first: ring FIFO drains all
            # of chi's descriptors before w2's start, so chi's completion is not
            # delayed by packet round-robin with the big transfer.
            t_chi = pool.tile([N, NB], f32)
            nc.sync.dma_start(out=t_chi[:], in_=chi[:])
            t_w2 = pool.tile([128, NB2], fc)
            nc.sync.dma_start(out=t_w2[:], in_=w2[:])

            # PE warm-up: the HAM clock gate keeps the PE at 1.2 GHz until it has
            # been busy ~3.4us. The PE is otherwise idle while the chi DMA is in
            # flight, so burn that window on dummy matmuls to enter the real
            # work at 2.4 GHz.
            if NWARM:
                t_garb = pool.tile([128, 128], f32)
                nc.vector.memset(t_garb[:], 1.0)
                ps_warm = psum.tile([128, 128], f32)
                for _ in range(NWARM):
                    nc.tensor.matmul(
                        ps_warm[:], lhsT=t_garb[:], rhs=t_garb[:], start=True, stop=True
                    )

            # constants built on-device
            t_fill = pool.tile([128, 128], f32)
            nc.gpsimd.memset(t_fill[:], 1.0)
            t_identc = pool.tile([128, 128], fc)
            t_fillc = pool.tile([128, 128], fc)
            nc.gpsimd.memset(t_fillc[:], 1.0)
            nc.gpsimd.affine_select(
                out=t_identc[:],
                in_=t_fillc[:],
                pattern=[[1, 128]],
                compare_op=mybir.AluOpType.is_equal,
                fill=0.0,
                base=0,
                channel_multiplier=-1,
            )
            # ONES - I: contracting chi with this directly yields the
            # leave-one-out column sums u[i,k] = sum_{j != i} chi[j,k]
            t_offdiag = pool.tile([128, 128], f32)
            nc.gpsimd.affine_select(
                out=t_offdiag[:],
                in_=t_fill[:],
                pattern=[[1, 128]],
                compare_op=mybir.AluOpType.not_equal,
                fill=0.0,
                base=0,
                channel_multiplier=-1,
            )

            # t_vr[i', a*32 + l] = chi[i', l]  (chi tiled 4x along free)
            t_vr = pool.tile([N, 128], fc)
            nc.scalar.activation(
                t_vr[:, :].rearrange("p (a l) -> p a l", a=4),
                t_chi[:, :].rearrange("p (one l) -> p one l", one=1).broadcast_to([N, 4, NB]),
                mybir.ActivationFunctionType.Copy,
            )

            # u[i, k] = sum_{j != i} chi[j, k]  (one matmul; offdiag is symmetric)
            ps_u = psum.tile([N, NB], f32)
            nc.tensor.matmul(ps_u[:], lhsT=t_offdiag[:], rhs=t_chi[:], start=True, stop=True)

            # vrep[p, i] = chi[i, p & 31]  (PE transpose mode: fp32 single pass)
            ps_vrep = psum.tile([128, 128], fc)
            nc.tensor.transpose(ps_vrep[:], t_vr[:], t_identc[:])
            t_vrep = pool.tile([128, 128], fc)
            nc.scalar.activation(t_vrep[:], ps_vrep[:], mybir.ActivationFunctionType.Copy)

            # chunk-level pipeline: per chunk c
            #   t_ur_c[i', 32t + r] = u[i', 4c + t]      (DVE/ACT stride-0 copy)
            #   urep_c = t_ur_c.T                        (PE transpose, ping-pong PSUM)
            #   bT_c = urep_c * vrep                     (DVE)
            #   ps_out += bT_c.T @ w2_c                  (PE, accumulating)
            t_ur = pool.tile([N, NB2], fc)
            t_bT = pool.tile([128, NB2], fc)
            ps_out = psum.tile([NORB, N], f32)
            ps_pingA = psum.tile([128, 256], fc, name="ps_pingA")
            ps_pingB = psum.tile([128, 256], fc, name="ps_pingB")
            ps_ping = [ps_pingA, ps_pingB]
            # groups of chunks: copy+transpose each chunk, then one TT and the
            # group's accumulating matmuls (fewer DVE ops and semaphore hops).
            # First group is a single chunk so the accumulation starts earliest.
            groups = [(0, 2), (2, 2), (4, 2), (6, 2)]
            for g, (c0, glen) in enumerate(groups):
                ps_g = ps_ping[g % 2]
                for h in range(glen):
                    c = c0 + h
                    ur_c = t_ur[:, c * 128 : (c + 1) * 128]
                    src_c = (
                        ps_u[:, 4 * c : 4 * c + 4]
                        .rearrange("p (j one) -> p j one", one=1)
                        .broadcast_to([N, 4, 32])
                    )
                    if COPY_ENGINE == "alt" and c % 2 == 1:
                        nc.scalar.activation(
                            ur_c.rearrange("p (j r) -> p j r", j=4),
                            src_c,
                            mybir.ActivationFunctionType.Copy,
                        )
                    else:
                        nc.vector.tensor_copy(ur_c.rearrange("p (j r) -> p j r", j=4), src_c)
                    nc.tensor.transpose(ps_g[:, h * 128 : (h + 1) * 128], ur_c, t_identc[:])
                bT_g = t_bT[:, c0 * 128 : (c0 + glen) * 128]
                nc.vector.tensor_mul(
                    bT_g.rearrange("p (h i) -> p h i", h=glen),
                    ps_g[:, : glen * 128].rearrange("p (h i) -> p h i", h=glen),
                    t_vrep[:, :]
                    .rearrange("p (one i) -> p one i", one=1)
                    .broadcast_to([128, glen, 128]),
                )
                for h in range(glen):
                    c = c0 + h
                    # stationary = w2 chunk (ready early, LDWEIGHTS pulled ahead
                    # by the PE reorder window); moving = basis chunk. Output is
                    # [o, i]; the host un-transposes during the gather.
                    nc.tensor.matmul(
                        ps_out[:],
                        lhsT=t_w2[:, c * NORB : (c + 1) * NORB],
                        rhs=t_bT[:, c * 128 : (c + 1) * 128],
                        start=(c == 0),
                        stop=(c == NCHUNK - 1),
                    )
            t_out = pool.tile([NORB, N], f32)
            nc.vector.tensor_copy(t_out[:], ps_out[:])
            nc.sync.dma_start(out=y[:], in_=t_out[:])

    nc.compile()
    return nc


def get_nc():
    if "nc" not in _cache:
        _cache["nc"] = _build()
    return _cache["nc"]


def make_in_maps(chi, backflow_coeff):
    chi = np.ascontiguousarray(chi, dtype=np.float32)
    w = np.ascontiguousarray(backflow_coeff, dtype=np.float32)
    assert chi.shape == (B, N, NB), chi.shape
    assert w.shape == (NB2, NORB), w.shape
    w2 = (w / np.float32(N - 1)).reshape(NCHUNK, 128, NORB).transpose(1, 0, 2)
    w2 = w2.reshape(128, NCHUNK * NORB)
    if CONTRACT == "fp16":
        w2 = w2.astype(np.float16)
    w2 = np.ascontiguousarray(w2)
    return [{"chi": chi[b], "w2": w2} for b in range(B)]


def kernel(chi, backflow_coeff):
    from concourse.bass_utils import run_bass_kernel_spmd

    nc = get_nc()
    in_maps = make_in_maps(chi, backflow_coeff)
    res = run_bass_kernel_spmd(nc, in_maps, list(range(B)))
    return np.ascontiguousarray(
        np.stack([res.results[b]["y"].T for b in range(B)])
    )


# revision 35
# speedup vs baseline: 1.0074x; 1.0074x over previous
"""Trainium2 Bass kernel for BasisOrbitalBackflow.

Math: for each batch b,
    basis[i, k*NB+l] = (1/(N-1)) * (u[i,k]) * chi[i,l],  u[i,k] = sum_{j != i} chi[j,k]
    out = basis @ W
(the mean over j != i of the pair outer product chi[j,k]*chi[i,l] collapses to
an outer product of the leave-one-out column sum with the row itself, so the
[B, N, N, NB^2] pair tensor is never materialized).

Sharding: data parallel, batch b -> core b (B == n_cores == 8).

Per-core dataflow (all shapes [partition, free]):
  u = (ONES - I) @ chi                                  (PE; one [128,128]x[128,32] matmul)
  t_ur[i, j*32+r] = u[i, j]                             (DVE/ACT stride-0 broadcast copies)
  urep_c[p, i] = t_ur_c.T = u[i, 4c + p>>5]             (PE transpose mode, fp32 single pass)
  vrep[p, i] = chi[i, p & 31]                           (same: tile 4x then PE transpose)
  bT[p, c*128+i] = urep_c[p, i] * vrep[p, i]            (DVE multiplies, chunk-pipelined)
  out[i, o] = sum_c sum_p bT_c[p, i] * w2[p, c*128+o]   (PE: 8 accumulating matmuls)
where w2[p, c*128+o] = W[c*128+p, o] / (N-1) is a host-side repack of
backflow_coeff (free: weights are static), and ONES-I / identity are built
on-device via memset + affine_select.
"""

import numpy as np

B, N, NB, NORB = 8, 128, 32, 128
NWARM = 0
# "fp32": full precision everywhere (rel err ~1e-6).
# "fp16": basis/W contraction in fp16 (single-pass PE, FWL) — rel err ~5e-4.
CONTRACT = "fp32"
COPY_ENGINE = "alt"

NB2 = NB * NB  # 1024
NCHUNK = NB2 // 128  # 8

_cache = {}


def _build():
    import concourse.bacc as bacc
    import concourse.mybir as mybir
    from concourse.tile import TileContext

    f32 = mybir.dt.float32
    fc = mybir.dt.float16 if CONTRACT == "fp16" else f32
    nc = bacc.Bacc(None, target_bir_lowering=False, debug=False, num_devices=8)
    chi = nc.declare_dram_parameter("chi", [N, NB], f32, isOutput=False)
    w2 = nc.declare_dram_parameter("w2", [128, NB2], fc, isOutput=False)
    y = nc.declare_dram_parameter("y", [N, NORB], f32, isOutput=True)

    with TileContext(nc) as tc:
        with (
            tc.tile_pool(name="sb", bufs=1) as pool,
            tc.tile_pool(name="ps", bufs=1, space="PSUM") as psum,
        ):
            # both loads on the sync HWDGE ring, chi first: ring FIFO drains all
            # of chi's descriptors before w2's start, so chi's completion is not
            # delayed by packet round-robin with the big transfer.
            t_chi = pool.tile([N, NB], f32)
            nc.sync.dma_start(out=t_chi[:], in_=chi[:])
            t_w2 = pool.tile([128, NB2], fc)
            nc.sync.dma_start(out=t_w2[:], in_=w2[:])

            # PE warm-up: the HAM clock gate keeps the PE at 1.2 GHz until it has
            # been busy ~3.4us. The PE is otherwise idle while the chi DMA is in
            # flight, so burn that window on dummy matmuls to enter the real
            # work at 2.4 GHz.
            if NWARM:
                t_garb = pool.tile([128, 128], f32)
                nc.vector.memset(t_garb[:], 1.0)
                ps_warm = psum.tile([128, 128], f32)
                for _ in range(NWARM):
                    nc.tensor.matmul(
                        ps_warm[:], lhsT=t_garb[:], rhs=t_garb[:], start=True, stop=True
                    )

            # constants built on-device
            t_fill = pool.tile([128, 128], f32)
            nc.gpsimd.memset(t_fill[:], 1.0)
            t_identc = pool.tile([128, 128], fc)
            t_fillc = pool.tile([128, 128], fc)
            nc.gpsimd.memset(t_fillc[:], 1.0)
            nc.gpsimd.affine_select(
                out=t_identc[:],
                in_=t_fillc[:],
                pattern=[[1, 128]],
                compare_op=mybir.AluOpType.is_equal,
                fill=0.0,
                base=0,
                channel_multiplier=-1,
            )
            # ONES - I: contracting chi with this directly yields the
            # leave-one-out column sums u[i,k] = sum_{j != i} chi[j,k]
            t_offdiag = pool.tile([128, 128], f32)
            nc.gpsimd.affine_select(
                out=t_offdiag[:],
                in_=t_fill[:],
                pattern=[[1, 128]],
                compare_op=mybir.AluOpType.not_equal,
                fill=0.0,
                base=0,
                channel_multiplier=-1,
            )

            # t_vr[i', a*32 + l] = chi[i', l]  (chi tiled 4x along free)
            t_vr = pool.tile([N, 128], fc)
            nc.scalar.activation(
                t_vr[:, :].rearrange("p (a l) -> p a l", a=4),
                t_chi[:, :].rearrange("p (one l) -> p one l", one=1).broadcast_to([N, 4, NB]),
                mybir.ActivationFunctionType.Copy,
            )

            # u[i, k] = sum_{j != i} chi[j, k]  (one matmul; offdiag is symmetric)
            ps_u = psum.tile([N, NB], f32)
            nc.tensor.matmul(ps_u[:], lhsT=t_offdiag[:], rhs=t_chi[:], start=True, stop=True)

            # vrep[p, i] = chi[i, p & 31]  (PE transpose mode: fp32 single pass)
            ps_vrep = psum.tile([128, 128], fc)
            nc.tensor.transpose(ps_vrep[:], t_vr[:], t_identc[:])
            t_vrep = pool.tile([128, 128], fc)
            nc.scalar.activation(t_vrep[:], ps_vrep[:], mybir.ActivationFunctionType.Copy)

            # chunk-level pipeline: per chunk c
            #   t_ur_c[i', 32t + r] = u[i', 4c + t]      (DVE/ACT stride-0 copy)
            #   urep_c = t_ur_c.T                        (PE transpose, ping-pong PSUM)
            #   bT_c = urep_c * vrep                     (DVE)
            #   ps_out += bT_c.T @ w2_c                  (PE, accumulating)
            t_ur = pool.tile([N, NB2], fc)
            t_bT = pool.tile([128, NB2], fc)
            ps_out = psum.tile([N, NORB], f32)
            ps_pingA = psum.tile([128, 256], fc, name="ps_pingA")
            ps_pingB = psum.tile([128, 256], fc, name="ps_pingB")
            ps_ping = [ps_pingA, ps_pingB]
            # groups of chunks: copy+transpose each chunk, then one TT and the
            # group's accumulating matmuls (fewer DVE ops and semaphore hops).
            # First group is a single chunk so the accumulation starts earliest.
            groups = [(0, 2), (2, 2), (4, 2), (6, 2)]
            for g, (c0, glen) in enumerate(groups):
                ps_g = ps_ping[g % 2]
                for h in range(glen):
                    c = c0 + h
                    ur_c = t_ur[:, c * 128 : (c + 1) * 128]
                    src_c = (
                        ps_u[:, 4 * c : 4 * c + 4]
                        .rearrange("p (j one) -> p j one", one=1)
                        .broadcast_to([N, 4, 32])
                    )
                    if COPY_ENGINE == "alt" and c % 2 == 1:
                        nc.scalar.activation(
                            ur_c.rearrange("p (j r) -> p j r", j=4),
                            src_c,
                            mybir.ActivationFunctionType.Copy,
                        )
                    else:
                        nc.vector.tensor_copy(ur_c.rearrange("p (j r) -> p j r", j=4), src_c)
                    nc.tensor.transpose(ps_g[:, h * 128 : (h + 1) * 128], ur_c, t_identc[:])
                bT_g = t_bT[:, c0 * 128 : (c0 + glen) * 128]
                nc.vector.tensor_mul(
                    bT_g.rearrange("p (h i) -> p h i", h=glen),
                    ps_g[:, : glen * 128].rearrange("p (h i) -> p h i", h=glen),
                    t_vrep[:, :]
                    .rearrange("p (one i) -> p one i", one=1)
                    .broadcast_to([128, glen, 128]),
                )
                for h in range(glen):
                    c = c0 + h
                    nc.tensor.matmul(
                        ps_out[:],
                        lhsT=t_bT[:, c * 128 : (c + 1) * 128],
                        rhs=t_w2[:, c * NORB : (c + 1) * NORB],
                        start=(c == 0),
                        stop=(c == NCHUNK - 1),
                    )
            t_out = pool.tile([N, NORB], f32)
            nc.vector.tensor_copy(t_out[:], ps_out[:])
            nc.sync.dma_start(out=y[:], in_=t_out[:])

    nc.compile()
    return nc


def get_nc():
    if "nc" not in _cache:
        _cache["nc"] = _build()
    return _cache["nc"]


def make_in_maps(chi, backflow_coeff):
    chi = np.ascontiguousarray(chi, dtype=np.float32)
    w = np.ascontiguousarray(backflow_coeff, dtype=np.float32)
    assert chi.shape == (B, N, NB), chi.shape
    assert w.shape == (NB2, NORB), w.shape
    w2 = (w / np.float32(N - 1)).reshape(NCHUNK, 128, NORB).transpose(1, 0, 2)
    w2 = w2.reshape(128, NCHUNK * NORB)
    if CONTRACT == "fp16":
        w2 = w2.astype(np.float16)
    w2 = np.ascontiguousarray(w2)
    return [{"chi": chi[b], "w2": w2} for b in range(B)]


def kernel(chi, backflow_coeff):
    from concourse.bass_utils import run_bass_kernel_spmd

    nc = get_nc()
    in_maps = make_in_maps(chi, backflow_coeff)
    res = run_bass_kernel_spmd(nc, in_maps, list(range(B)))
    return np.stack([res.results[b]["y"] for b in range(B)])


# revision 36
# speedup vs baseline: 1.0285x; 1.0209x over previous
"""Trainium2 Bass kernel for BasisOrbitalBackflow.

Math: for each batch b,
    basis[i, k*NB+l] = (1/(N-1)) * (u[i,k]) * chi[i,l],  u[i,k] = sum_{j != i} chi[j,k]
    out = basis @ W
(the mean over j != i of the pair outer product chi[j,k]*chi[i,l] collapses to
an outer product of the leave-one-out column sum with the row itself, so the
[B, N, N, NB^2] pair tensor is never materialized).

Sharding: data parallel, batch b -> core b (B == n_cores == 8).

Per-core dataflow (all shapes [partition, free]):
  u = (ONES - I) @ chi                                  (PE; one [128,128]x[128,32] matmul)
  t_ur[i, j*32+r] = u[i, j]                             (DVE/ACT stride-0 broadcast copies)
  urep_c[p, i] = t_ur_c.T = u[i, 4c + p>>5]             (PE transpose mode, fp32 single pass)
  vrep[p, i] = chi[i, p & 31]                           (same: tile 4x then PE transpose)
  bT[p, c*128+i] = urep_c[p, i] * vrep[p, i]            (DVE multiplies, chunk-pipelined)
  out[i, o] = sum_c sum_p bT_c[p, i] * w2[p, c*128+o]   (PE: 8 accumulating matmuls)
where w2[p, c*128+o] = W[c*128+p, o] / (N-1) is a host-side repack of
backflow_coeff (free: weights are static), and ONES-I / identity are built
on-device via memset + affine_select.
"""

import numpy as np

B, N, NB, NORB = 8, 128, 32, 128
NWARM = 0
# "fp32": full precision everywhere (rel err ~1e-6).
# "fp16": basis/W contraction in fp16 (single-pass PE, FWL) — rel err ~5e-4.
CONTRACT = "fp32"
COPY_ENGINE = "alt"

NB2 = NB * NB  # 1024
NCHUNK = NB2 // 128  # 8

_cache = {}


def _build():
    import concourse.bacc as bacc
    import concourse.mybir as mybir
    from concourse.tile import TileContext

    f32 = mybir.dt.float32
    fc = mybir.dt.float16 if CONTRACT == "fp16" else f32
    nc = bacc.Bacc(None, target_bir_lowering=False, debug=False, num_devices=8)
    chi = nc.declare_dram_parameter("chi", [N, NB], f32, isOutput=False)
    w2 = nc.declare_dram_parameter("w2", [128, NB2], fc, isOutput=False)
    y = nc.declare_dram_parameter("y", [N, NORB], f32, isOutput=True)

    with TileContext(nc) as tc:
        with (
            tc.tile_pool(name="sb", bufs=1) as pool,
            tc.tile_pool(name="ps", bufs=1, space="PSUM") as psum,
        ):
            # both loads on the sync HWDGE ring, chi first: ring FIFO drains all
            # of chi's descriptors before w2's start, so chi's completion is not
            # delayed by packet round-robin with the big transfer.
            t_chi = pool.tile([N, NB], f32)
            nc.sync.dma_start(out=t_chi[:], in_=chi[:])
            t_w2 = pool.tile([128, NB2], fc)
            nc.sync.dma_start(out=t_w2[:], in_=w2[:])

            # PE warm-up: the HAM clock gate keeps the PE at 1.2 GHz until it has
            # been busy ~3.4us. The PE is otherwise idle while the chi DMA is in
            # flight, so burn that window on dummy matmuls to enter the real
            # work at 2.4 GHz.
            if NWARM:
                t_garb = pool.tile([128, 128], f32)
                nc.vector.memset(t_garb[:], 1.0)
                ps_warm = psum.tile([128, 128], f32)
                for _ in range(NWARM):
                    nc.tensor.matmul(
                        ps_warm[:], lhsT=t_garb[:], rhs=t_garb[:], start=True, stop=True
                    )

            # constants built on-device
            t_fill = pool.tile([128, 128], f32)
            nc.gpsimd.memset(t_fill[:], 1.0)
            t_identc = pool.tile([128, 128], fc)
            t_fillc = pool.tile([128, 128], fc)
            nc.gpsimd.memset(t_fillc[:], 1.0)
            nc.gpsimd.affine_select(
                out=t_identc[:],
                in_=t_fillc[:],
                pattern=[[1, 128]],
                compare_op=mybir.AluOpType.is_equal,
                fill=0.0,
                base=0,
                channel_multiplier=-1,
            )
            # ONES - I: contracting chi with this directly yields the
            # leave-one-out column sums u[i,k] = sum_{j != i} chi[j,k]
            t_offdiag = pool.tile([128, 128], f32)
            nc.gpsimd.affine_select(
                out=t_offdiag[:],
                in_=t_fill[:],
                pattern=[[1, 128]],
                compare_op=mybir.AluOpType.not_equal,
                fill=0.0,
                base=0,
                channel_multiplier=-1,
            )

            # t_vr[i', a*32 + l] = chi[i', l]  (chi tiled 4x along free)
            t_vr = pool.tile([N, 128], fc)
            nc.scalar.activation(
                t_vr[:, :].rearrange("p (a l) -> p a l", a=4),
                t_chi[:, :].rearrange("p (one l) -> p one l", one=1).broadcast_to([N, 4, NB]),
                mybir.ActivationFunctionType.Copy,
            )

            # u[i, k] = sum_{j != i} chi[j, k]  (one matmul; offdiag is symmetric)
            ps_u = psum.tile([N, NB], f32)
            nc.tensor.matmul(ps_u[:], lhsT=t_offdiag[:], rhs=t_chi[:], start=True, stop=True)

            # vrep[p, i] = chi[i, p & 31]  (PE transpose mode: fp32 single pass)
            ps_vrep = psum.tile([128, 128], fc)
            nc.tensor.transpose(ps_vrep[:], t_vr[:], t_identc[:])
            t_vrep = pool.tile([128, 128], fc)
            nc.scalar.activation(t_vrep[:], ps_vrep[:], mybir.ActivationFunctionType.Copy)

            # chunk-level pipeline: per chunk c
            #   t_ur_c[i', 32t + r] = u[i', 4c + t]      (DVE/ACT stride-0 copy)
            #   urep_c = t_ur_c.T                        (PE transpose, ping-pong PSUM)
            #   bT_c = urep_c * vrep                     (DVE)
            #   ps_out += bT_c.T @ w2_c                  (PE, accumulating)
            t_ur = pool.tile([N, NB2], fc)
            t_bT = pool.tile([128, NB2], fc)
            ps_out = psum.tile([N, NORB], f32)
            ps_pingA = psum.tile([128, 256], fc, name="ps_pingA")
            ps_pingB = psum.tile([128, 256], fc, name="ps_pingB")
            ps_ping = [ps_pingA, ps_pingB]
            # groups of chunks: copy+transpose each chunk, then one TT and the
            # group's accumulating matmuls (fewer DVE ops and semaphore hops).
            # First group is a single chunk so the accumulation starts earliest.
            groups = [(0, 2), (2, 2), (4, 2), (6, 2)]

            def emit_mms(c0, glen):
                for h in range(glen):
                    c = c0 + h
                    nc.tensor.matmul(
                        ps_out[:],
                        lhsT=t_bT[:, c * 128 : (c + 1) * 128],
                        rhs=t_w2[:, c * NORB : (c + 1) * NORB],
                        start=(c == 0),
                        stop=(c == NCHUNK - 1),
                    )

            # Emit each group's matmuls AFTER the next group's transposes, so
            # the PE never runs accumulation matmuls ahead of the transposes
            # that feed the vector engine (otherwise the last basis chunks
            # starve and the final matmuls stall ~1.5us).
            prev_group = None
            for g, (c0, glen) in enumerate(groups):
                ps_g = ps_ping[g % 2]
                for h in range(glen):
                    c = c0 + h
                    ur_c = t_ur[:, c * 128 : (c + 1) * 128]
                    src_c = (
                        ps_u[:, 4 * c : 4 * c + 4]
                        .rearrange("p (j one) -> p j one", one=1)
                        .broadcast_to([N, 4, 32])
                    )
                    if COPY_ENGINE == "alt" and c % 2 == 1:
                        nc.scalar.activation(
                            ur_c.rearrange("p (j r) -> p j r", j=4),
                            src_c,
                            mybir.ActivationFunctionType.Copy,
                        )
                    else:
                        nc.vector.tensor_copy(ur_c.rearrange("p (j r) -> p j r", j=4), src_c)
                    nc.tensor.transpose(ps_g[:, h * 128 : (h + 1) * 128], ur_c, t_identc[:])
                if prev_group is not None:
                    emit_mms(*prev_group)
                bT_g = t_bT[:, c0 * 128 : (c0 + glen) * 128]
                nc.vector.tensor_mul(
                    bT_g.rearrange("p (h i) -> p h i", h=glen),
                    ps_g[:, : glen * 128].rearrange("p (h i) -> p h i", h=glen),
                    t_vrep[:, :]
                    .rearrange("p (one i) -> p one i", one=1)
                    .broadcast_to([128, glen, 128]),
                )
                prev_group = (c0, glen)
            emit_mms(*prev_group)
            t_out = pool.tile([N, NORB], f32)
            nc.vector.tensor_copy(t_out[:], ps_out[:])
            nc.sync.dma_start(out=y[:], in_=t_out[:])

    nc.compile()
    return nc


def get_nc():
    if "nc" not in _cache:
        _cache["nc"] = _build()
    return _cache["nc"]


def make_in_maps(chi, backflow_coeff):
    chi = np.ascontiguousarray(chi, dtype=np.float32)
    w = np.ascontiguousarray(backflow_coeff, dtype=np.float32)
    assert chi.shape == (B, N, NB), chi.shape
    assert w.shape == (NB2, NORB), w.shape
    w2 = (w / np.float32(N - 1)).reshape(NCHUNK, 128, NORB).transpose(1, 0, 2)
    w2 = w2.reshape(128, NCHUNK * NORB)
    if CONTRACT == "fp16":
        w2 = w2.astype(np.float16)
    w2 = np.ascontiguousarray(w2)
    return [{"chi": chi[b], "w2": w2} for b in range(B)]


def kernel(chi, backflow_coeff):
    from concourse.bass_utils import run_bass_kernel_spmd

    nc = get_nc()
    in_maps = make_in_maps(chi, backflow_coeff)
    res = run_bass_kernel_spmd(nc, in_maps, list(range(B)))
    return np.stack([res.results[b]["y"] for b in range(B)])
